# revision 1
# baseline (speedup 1.0000x reference)
"""Trainium2 Bass kernel for nn_ContrastiveLoss (8-core SPMD).

Strategy (memory-bound gather problem):
  - Shard the 262144 pos + 262144 neg pairs across 8 NeuronCores
    (32768 + 32768 pairs per core); replicate Xemb (4MB, stays in HBM).
  - Per core, per batch of 2048 pairs: two SWDGE dma_gather calls pull the
    512B embedding rows for i and j into SBUF [128, 16, 128] tiles (row t of
    the batch lands on partition t%128, chunk t//128).
  - DVE: diff = A - B.  ACT: square (for pos batches with fused per-partition
    accumulation -> one f32 per batch per partition).  DVE: per-pair reduce
    of the squared diff over D=128 for neg batches.
  - Device output per core: [128, 272] f32 = 16 pos batch-accum columns +
    256 per-pair d^2 columns for the 32768 neg pairs.
  - Host: f64 final math - pos mean, and relu(softplus(h_bias) - sqrt(d^2))^2
    mean for neg (elementwise, order-independent, so layout doesn't matter).
"""

import sys

if "/opt/trn_rl_repo" not in sys.path:
    sys.path.insert(0, "/opt/trn_rl_repo")

import numpy as np

import concourse.bass as bass
import concourse.mybir as mybir
from concourse.library_config import mlp
from concourse.library_overlay import lower_extended_insts
from concourse.bass_utils import run_bass_kernel_spmd

# Problem shapes (hardcoded per the harness contract).
N, D = 8192, 128
NUM_PAIRS = 262144
NCORES = 8
PAIRS = NUM_PAIRS // NCORES      # 32768 pairs per core per loss type
B = 1024                         # pairs per gather batch (1024 = single-packet max)
CH = B // 128                    # 16 chunks per batch
NB = PAIRS // B                  # 16 batches per loss type
NBT = 2 * NB                     # 32 batches total (pos then neg)
SLOTS_G = 4                      # gather tile slots (A/B pools)
SLOTS_C = 3                      # diff/square tile slots
R_COLS = NB + NB * CH            # 16 pos accum cols + 256 neg per-pair cols

_nc_cache = None
_last_results = None             # stashed BassKernelResults (for test harness)


def _build_nc(reps=1):
    nc = bass.Bass()
    xemb = nc.dram_tensor("xemb", [N, D], mybir.dt.float32, kind="ExternalInput")
    idx = nc.dram_tensor(
        "idx", [4, 128, PAIRS // 16], mybir.dt.int16, kind="ExternalInput"
    )
    out = nc.dram_tensor("out", [128, R_COLS], mybir.dt.float32, kind="ExternalOutput")

    with (
        nc.sbuf_tensor("idx_sb", [128, 4, PAIRS // 16], mybir.dt.int16) as idx_sb,
        nc.sbuf_tensor("ga", [128, SLOTS_G, CH, D], mybir.dt.float32) as ga,
        nc.sbuf_tensor("gb", [128, SLOTS_G, CH, D], mybir.dt.float32) as gb,
        nc.sbuf_tensor("df", [128, SLOTS_C, CH, D], mybir.dt.float32) as df,
        nc.sbuf_tensor("sq", [128, SLOTS_C, CH, D], mybir.dt.float32) as sq,
        nc.sbuf_tensor("acc", [128, R_COLS], mybir.dt.float32) as acc,
        nc.semaphore("s_idx") as s_idx,
        nc.semaphore("s_ga0") as s_ga0,
        nc.semaphore("s_ga1") as s_ga1,
        nc.semaphore("s_ga2") as s_ga2,
        nc.semaphore("s_ga3") as s_ga3,
        nc.semaphore("s_gb0") as s_gb0,
        nc.semaphore("s_gb1") as s_gb1,
        nc.semaphore("s_gb2") as s_gb2,
        nc.semaphore("s_gb3") as s_gb3,
        nc.semaphore("s_sub") as s_sub,
        nc.semaphore("s_act") as s_act,
        nc.semaphore("s_red") as s_red,
        nc.semaphore("s_out") as s_out,
        nc.Block() as block,
    ):
        s_ga = [s_ga0, s_ga1, s_ga2, s_ga3]
        s_gb = [s_gb0, s_gb1, s_gb2, s_gb3]

        @block.sync
        def _(sync):
            for plane in range(4):
                sync.dma_start(idx_sb[:, plane, :], idx[plane]).then_inc(s_idx, 16)
            sync.wait_ge(s_act, reps * NBT)
            sync.wait_ge(s_red, reps * NB)
            sync.dma_start(out[:], acc[:]).then_inc(s_out, 16)
            sync.wait_ge(s_out, 16)

        @block.gpsimd
        def _(gpsimd):
            gpsimd.load_library(mlp)
            nreg = gpsimd.to_reg(B)
            gpsimd.wait_ge(s_idx, 64)
            for kg in range(reps * NBT):
                k = kg % NBT
                s = kg % SLOTS_G
                loss, b = divmod(k, NB)
                ip, jp = 2 * loss, 2 * loss + 1
                cols = slice(b * (B // 16), (b + 1) * (B // 16))
                if kg >= SLOTS_G:
                    gpsimd.wait_ge(s_sub, kg - SLOTS_G + 1)
                gpsimd.dma_gather(
                    ga[:, s], xemb[:], idx_sb[:, ip, cols], B, nreg, D,
                ).then_inc(s_ga[s], 16)
                gpsimd.dma_gather(
                    gb[:, s], xemb[:], idx_sb[:, jp, cols], B, nreg, D,
                ).then_inc(s_gb[s], 16)
            del k, s, loss, b, ip, jp, cols

        @block.vector
        def _(vector):
            def emit_reduce(kgp):
                # per-pair reduce for neg batch kgp: [128, CH, D] -> [128, CH]
                sc = kgp % SLOTS_C
                bn = (kgp % NBT) - NB
                vector.wait_ge(s_act, kgp + 1)
                vector.tensor_reduce(
                    acc[:, NB + bn * CH : NB + (bn + 1) * CH],
                    sq[:, sc],
                    axis=mybir.AxisListType.X,
                    op=mybir.AluOpType.add,
                ).then_inc(s_red, 1)

            for kg in range(reps * NBT):
                k = kg % NBT
                s = kg % SLOTS_G
                sc = kg % SLOTS_C
                if kg >= SLOTS_C:
                    vector.wait_ge(s_act, kg - SLOTS_C + 1)
                vector.wait_ge(s_ga[s], 16 * (kg // SLOTS_G + 1))
                vector.wait_ge(s_gb[s], 16 * (kg // SLOTS_G + 1))
                vector.tensor_sub(df[:, sc], ga[:, s], gb[:, s]).then_inc(s_sub, 1)
                if (kg - 1) % NBT >= NB and kg >= 1:
                    emit_reduce(kg - 1)
            emit_reduce(reps * NBT - 1)

        @block.scalar
        def _(scalar):
            nred = 0
            for kg in range(reps * NBT):
                k = kg % NBT
                sc = kg % SLOTS_C
                scalar.wait_ge(s_sub, kg + 1)
                if kg >= SLOTS_C and (kg - SLOTS_C) % NBT >= NB:
                    nred += 1
                    scalar.wait_ge(s_red, nred)
                if k < NB:
                    scalar.activation(
                        sq[:, sc],
                        df[:, sc],
                        mybir.ActivationFunctionType.Square,
                        accum_out=acc[:, k : k + 1],
                    ).then_inc(s_act, 1)
                else:
                    scalar.activation(
                        sq[:, sc],
                        df[:, sc],
                        mybir.ActivationFunctionType.Square,
                    ).then_inc(s_act, 1)

    lower_extended_insts(nc)
    return nc


def _get_nc():
    global _nc_cache
    if _nc_cache is None:
        _nc_cache = _build_nc()
    return _nc_cache


def _wrap_idx(arr):
    """int32 [PAIRS] -> wrapped int16 [128, PAIRS//16] for dma_gather."""
    wrapped = arr.astype(np.int16).reshape(PAIRS // 16, 16).T  # [16, PAIRS//16]
    return np.tile(wrapped, (8, 1))


def kernel(**inputs):
    global _last_results
    Xemb = np.ascontiguousarray(np.asarray(inputs["Xemb"], dtype=np.float32))
    h_bias = float(np.asarray(inputs["h_bias"]))
    pos_idx = np.asarray(inputs["pos_idx"], dtype=np.int32)
    neg_idx = np.asarray(inputs["neg_idx"], dtype=np.int32)

    in_maps = []
    for c in range(NCORES):
        sl = slice(c * PAIRS, (c + 1) * PAIRS)
        planes = np.stack(
            [
                _wrap_idx(pos_idx[sl, 0]),
                _wrap_idx(pos_idx[sl, 1]),
                _wrap_idx(neg_idx[sl, 0]),
                _wrap_idx(neg_idx[sl, 1]),
            ]
        )
        in_maps.append({"xemb": Xemb, "idx": planes})

    res = run_bass_kernel_spmd(_get_nc(), in_maps, core_ids=list(range(NCORES)))
    _last_results = res

    pos_sum = 0.0
    neg_parts = []
    for c in range(NCORES):
        o = np.asarray(res.results[c]["out"], dtype=np.float64)
        pos_sum += o[:, :NB].sum()
        neg_parts.append(o[:, NB:])
    neg_sq = np.concatenate(neg_parts, axis=1).ravel()  # all 262144 neg d^2

    bias = np.logaddexp(0.0, h_bias)  # softplus, f64
    pos_loss = 0.5 * pos_sum / NUM_PAIRS
    d = np.sqrt(np.maximum(neg_sq, 0.0))
    m = np.maximum(bias - d, 0.0)
    neg_loss = 0.5 * np.mean(m * m)
    return np.array([pos_loss, neg_loss], dtype=np.float32)

